# revision 15
# baseline (speedup 1.0000x reference)
"""CondConv2D Trainium2 kernel (v2).

Problem (hardcoded): B=16, C_in=64, H=W=256, E=4, C_out=64, 3x3, s=1, d=1, p=1.
Sharding: data-parallel over batch, 8 cores x 2 images.

v2 changes vs v1:
  - fp32 HWDGE loads into a small staging pool; ACT does a fused
    fp32->bf16 cast + per-tile pooling reduction (activation accum_out),
    writing padded persistent bf16 tiles [128, 13, 258] (zero pad cols).
    No SWDGE cast DMAs, no DVE reduces.
  - All conv matmuls are full N=512 (edge columns come from the zero pad
    cols), removing the 255-col split matmuls that dominated v1.
  - 20 persistent image tiles (2 images x 10) - image i+1 loads/casts
    overlap image i's conv with no slot-rotation deadlocks.
  - Stores issued on the gpsimd (SWDGE) queue so they never queue behind
    loads (sync) or casts (scalar).
"""
import sys

if "/opt/trn_rl_repo" not in sys.path:
    sys.path.insert(0, "/opt/trn_rl_repo")

import numpy as np

import concourse.bacc as bacc
import concourse.mybir as mybir
import concourse.tile as tile
from concourse.bass_utils import run_bass_kernel_spmd

F32 = mybir.dt.float32
BF16 = mybir.dt.bfloat16
AF = mybir.ActivationFunctionType
ALU = mybir.AluOpType

N_CORES = 8
IMGS = 2
C_IN = 64
C_OUT = 64
H = 256
W = 256
E = 4
NTAP = 9
RPT = 13           # rows per tile
NT = 10            # tiles per image (130 rows per half: -1..128 / 127..256)
HALF = 128
STAGE_ROWS = 16


def build_nc():
    nc = bacc.Bacc("TRN2", target_bir_lowering=False, debug=False,
                   num_devices=N_CORES)
    # xp: host-prepared tile layout. Partition p<64: top-half channels,
    # row r = x row r-1 (row 0 = zero pad); p>=64: bottom-half channels,
    # row r = x row 127+r (row 129 = zero pad).
    x = nc.dram_tensor("xp", [IMGS, 128, 130, W], F32, kind="ExternalInput")
    wt = nc.dram_tensor("wt", [128, E * NTAP * C_OUT], F32,
                        kind="ExternalInput")
    fcw = nc.dram_tensor("fcw", [128, E], F32, kind="ExternalInput")
    fcb = nc.dram_tensor("fcb", [128, E], F32, kind="ExternalInput")
    ones = nc.dram_tensor("ones", [128, 128], F32, kind="ExternalInput")
    y = nc.dram_tensor("y", [IMGS, C_OUT, H, W], F32, kind="ExternalOutput")

    S = NTAP * C_OUT  # 576

    with tile.TileContext(nc) as tc:
        with (
            tc.tile_pool(name="consts", bufs=1) as consts,
            tc.tile_pool(name="stgp", bufs=2) as stgp,
            tc.tile_pool(name="small", bufs=2) as small,
            tc.tile_pool(name="stage", bufs=2) as stage_pool,
            tc.tile_pool(name="psum", bufs=1, space="PSUM") as psum_pool,
        ):
            # ---- consts ----
            wtmp = stgp.tile([128, E * S], F32, tag="stg",
                             padded_shape=[128, RPT * W])
            nc.sync.dma_start(wtmp[:], wt[:])
            wtb = consts.tile([128, E * S], BF16)
            nc.scalar.activation(wtb[:], wtmp[:], AF.Copy)
            fcwt = consts.tile([128, E], F32)
            fcbt = consts.tile([128, E], F32)
            onest = consts.tile([128, 128], F32)
            nc.sync.dma_start(fcwt[:], fcw[:])
            nc.sync.dma_start(fcbt[:], fcb[:])
            nc.sync.dma_start(onest[:], ones[:])

            # ---- persistent image tiles; memset pads once ----
            xs = [[consts.tile([128, RPT, 258], BF16, name=f"xs{i}_{t}")
                   for t in range(NT)] for i in range(IMGS)]
            for i in range(IMGS):
                for t in range(NT):
                    nc.vector.memset(xs[i][t][:, :, 0:1], 0.0)
                    nc.vector.memset(xs[i][t][:, :, 257:258], 0.0)
                # top half: row -1 pad; bottom half: row 256 pad
                nc.vector.memset(xs[i][0][0:64, 0:1, :], 0.0)
                nc.vector.memset(xs[i][NT - 1][64:128, 12:13, :], 0.0)

            # per-image routing partials (13 cast ops -> 13 cols used)
            partials = [small.tile([128, 16], F32, name=f"par{i}", tag="par",
                                   bufs=2) for i in range(IMGS)]
            for i in range(IMGS):
                nc.vector.memset(partials[i][:], 0.0)
            # zeros tile so DVE drains can use tensor_tensor (which never
            # enters the 2-port perf mode that starves SWDGE stores)
            zdrain = consts.tile([128, 2, W], F32, name="zdrain")
            nc.vector.memset(zdrain[:], 0.0)

            def load_image(i):
                par = partials[i]
                col = [0]

                def cast(dst_rows, src, hs, acc=True):
                    t_, r0, r1 = dst_rows
                    kw = {}
                    if acc:
                        kw["accum_out"] = par[hs, col[0]:col[0] + 1]
                        col[0] += 1
                    nc.scalar.activation(
                        xs[i][t_][hs, r0:r1, 1:257], src, AF.Copy, **kw)

                for t in range(NT):
                    stg = stgp.tile([128, RPT, W], F32, tag="stg")
                    # split halves across two DMA paths: partitions 0-63 hit
                    # the 8 even SBUF ports (sync/HWDGE ring), 64-127 the 8
                    # odd ports (gpsimd/SWDGE ring) - they drain in parallel.
                    nc.sync.dma_start(stg[0:64],
                                      x[i, 0:64, 13 * t:13 * t + 13, :])
                    nc.gpsimd.dma_start(stg[64:128],
                                        x[i, 64:128, 13 * t:13 * t + 13, :])
                    if t == 0:
                        cast((0, 0, 13), stg[0:64], slice(0, 64))
                        # bottom rows 0,1 are x rows 127,128, already counted
                        # by the top half - exclude from pooling accumulators.
                        cast((0, 0, 2), stg[64:128, 0:2, :], slice(64, 128),
                             acc=False)
                        cast((0, 2, 13), stg[64:128, 2:13, :], slice(64, 128))
                    else:
                        cast((t, 0, 13), stg[:], slice(0, 128))

            def routing(i):
                par = partials[i]
                pooled = small.tile([128, 1], F32, name="pooled")
                nc.vector.reduce_sum(pooled[:], par[:],
                                     axis=mybir.AxisListType.X)
                tmp4 = small.tile([128, E], F32, name="tmp4")
                nc.vector.tensor_scalar(tmp4[:], fcwt[:], pooled[:, 0:1],
                                        1.0 / float(H * W),
                                        op0=ALU.mult, op1=ALU.mult)
                ps4 = psum_pool.tile([128, E], F32, name="ps4", tag="rt",
                                     bufs=1)
                nc.tensor.matmul(ps4[0:64], onest[0:64, 0:64], tmp4[0:64],
                                 start=True, stop=True, tile_position=(0, 0),
                                 skip_group_check=True)
                nc.tensor.matmul(ps4[64:128], onest[64:128, 64:128],
                                 tmp4[64:128], start=True, stop=True,
                                 tile_position=(64, 64), skip_group_check=True)
                logits = small.tile([128, E], F32, name="logits")
                nc.vector.tensor_tensor(logits[:], ps4[:], fcbt[:], op=ALU.add)
                rt = small.tile([128, E], F32, name="rt")
                nc.scalar.activation(rt[:], logits[:], AF.Sigmoid)
                wmix = small.tile([128, S], BF16, name="wmix", tag="wmix")
                nc.vector.tensor_scalar_mul(wmix[:], wtb[:, 0:S], rt[:, 0:1])
                for e in range(1, E):
                    nc.vector.scalar_tensor_tensor(
                        wmix[:], wtb[:, e * S:(e + 1) * S], rt[:, e:e + 1],
                        wmix[:], op0=ALU.mult, op1=ALU.add)
                return wmix

            def conv(i, wmix):
                xi = xs[i]
                n_groups = 32           # 2 pairs per group
                gps = STAGE_ROWS // 4   # groups per stage tile (4)
                stage = None
                for g in range(n_groups):
                    if g % gps == 0:
                        stage = stage_pool.tile([128, STAGE_ROWS, W], F32,
                                                name="stage", tag="st")
                    psA = psum_pool.tile([128, 2, W], F32, name="psA",
                                         tag="ps", bufs=6)
                    psB = psum_pool.tile([128, 2, W], F32, name="psB",
                                         tag="ps", bufs=6)
                    pstiles = (psA, psB)
                    # last tap must be unsplit for both pairs: pick clean kh
                    bad = set()
                    for px in range(2):
                        pair = 2 * g + px
                        for kh in range(3):
                            if (2 * pair + kh) % RPT == RPT - 1:
                                bad.add(kh)
                    clean = [kh for kh in range(3) if kh not in bad][-1]
                    khs = [kh for kh in range(3) if kh != clean] + [clean]
                    taps = [kh * 3 + kw for kh in khs for kw in range(3)]
                    for r, tap in enumerate(taps):
                        kh, kw = divmod(tap, 3)
                        st = r == 0
                        sp = r == len(taps) - 1
                        for px in range(2):
                            pair = 2 * g + px
                            L = 2 * pair + kh
                            t, m = divmod(L, RPT)
                            ps = pstiles[px]
                            for half in range(2):
                                hs = slice(0, 64) if half == 0 else \
                                    slice(64, 128)
                                lhsT = wmix[hs, tap * 64:(tap + 1) * 64]
                                if px == 0:
                                    tp = (0, 0) if half == 0 else (64, 64)
                                    osl = hs
                                else:
                                    tp = (0, 64) if half == 0 else (64, 0)
                                    osl = slice(64, 128) if half == 0 else \
                                        slice(0, 64)
                                if m <= RPT - 2:
                                    rhs = xi[t][hs, m:m + 2, kw:kw + 256]
                                    nc.tensor.matmul(
                                        ps[osl], lhsT, rhs, start=st, stop=sp,
                                        tile_position=tp,
                                        skip_group_check=True)
                                else:
                                    for j in range(2):
                                        tj, mj = divmod(L + j, RPT)
                                        rhs = xi[tj][hs, mj, kw:kw + 256]
                                        nc.tensor.matmul(
                                            ps[osl, j, :], lhsT, rhs,
                                            start=(st and j == 0), stop=sp,
                                            tile_position=tp,
                                            skip_group_check=True)
                    # drain psum -> staging (alternate DVE / ACT)
                    r0 = (g % gps) * 4
                    for px in range(2):
                        dst = stage[:, r0 + 2 * px:r0 + 2 * px + 2, :]
                        src = pstiles[px][:]
                        # all drains on DVE: the ACT queue is busy with the
                        # next image's casts during conv, and tensor_tensor
                        # never enters the 2-port mode that starves SWDGE.
                        nc.vector.tensor_tensor(dst, src, zdrain[:],
                                                op=ALU.add)
                    # stage full -> 4 interleaved store DMAs on gpsimd
                    if (g + 1) % gps == 0:
                        mrow = (g // gps) * STAGE_ROWS
                        nj = STAGE_ROWS // 4
                        sv = stage.rearrange("p (j b r) w -> p j b r w",
                                             j=nj, b=2, r=2)
                        ys = y[i].rearrange("c (blk four) w -> c blk four w",
                                            four=4)
                        nc.gpsimd.dma_start(
                            ys[:, mrow // 4:mrow // 4 + nj, 0:2, :],
                            sv[0:64, :, 0, :, :])
                        nc.gpsimd.dma_start(
                            ys[:, (HALF + mrow) // 4:(HALF + mrow) // 4 + nj,
                               2:4, :],
                            sv[0:64, :, 1, :, :])
                        nc.gpsimd.dma_start(
                            ys[:, (HALF + mrow) // 4:(HALF + mrow) // 4 + nj,
                               0:2, :],
                            sv[64:128, :, 0, :, :])
                        nc.gpsimd.dma_start(
                            ys[:, mrow // 4:mrow // 4 + nj, 2:4, :],
                            sv[64:128, :, 1, :, :])

            # ---- schedule ----
            # routing(0) is emitted before load_image(1) so image 0's
            # routing ops are not stuck behind image 1's casts in the
            # ACT/DVE FIFO queues.
            load_image(0)
            wmix0 = routing(0)
            load_image(1)
            conv(0, wmix0)
            wmix1 = routing(1)
            conv(1, wmix1)

    nc.compile()
    return nc


_NC_CACHE = {}


def _get_nc():
    if "nc" not in _NC_CACHE:
        _NC_CACHE["nc"] = build_nc()
    return _NC_CACHE["nc"]


def _prep_x(x2):
    """[2, 64, 256, 256] -> tile layout [2, 128, 130, 256] (pads baked)."""
    xp = np.zeros((IMGS, 128, 130, W), dtype=np.float32)
    xp[:, 0:64, 1:130, :] = x2[:, :, 0:129, :]
    xp[:, 64:128, 0:129, :] = x2[:, :, 127:256, :]
    return xp


def _prep_shared(weight, fc_w, fc_b):
    # [E, O, I, KH, KW] -> [I, E, KH, KW, O] -> [64, E*9*64], dup halves
    wt = np.ascontiguousarray(weight.transpose(2, 0, 3, 4, 1)).reshape(
        C_IN, E * NTAP * C_OUT)
    wt = np.concatenate([wt, wt], axis=0).astype(np.float32)
    fcw = np.concatenate([fc_w.T, fc_w.T], axis=0).astype(np.float32)
    fcb = np.tile(fc_b.reshape(1, E), (128, 1)).astype(np.float32)
    ones = np.ones((128, 128), np.float32)
    return wt, fcw, fcb, ones


def kernel(inputs, weight, fc_w, fc_b, stride=1, dilation=1, padding=1,
           _trace=False, _npx=2):
    assert int(stride) == 1 and int(dilation) == 1 and int(padding) == 1
    inputs = np.asarray(inputs, dtype=np.float32)
    B = inputs.shape[0]
    assert B == N_CORES * IMGS
    wt, fcw, fcb, ones = _prep_shared(np.asarray(weight), np.asarray(fc_w),
                                      np.asarray(fc_b))
    nc = _get_nc()
    in_maps = []
    for c in range(N_CORES):
        in_maps.append({
            "xp": _prep_x(inputs[2 * c:2 * c + 2]),
            "wt": wt, "fcw": fcw, "fcb": fcb, "ones": ones,
        })
    res = run_bass_kernel_spmd(nc, in_maps, core_ids=list(range(N_CORES)),
                               trace=_trace)
    out = np.concatenate([res.results[c]["y"] for c in range(N_CORES)], axis=0)
    if _trace:
        return out, res
    return out


# revision 16
# speedup vs baseline: 1.1491x; 1.1491x over previous
"""CondConv2D Trainium2 kernel (v2).

Problem (hardcoded): B=16, C_in=64, H=W=256, E=4, C_out=64, 3x3, s=1, d=1, p=1.
Sharding: data-parallel over batch, 8 cores x 2 images.

v2 changes vs v1:
  - fp32 HWDGE loads into a small staging pool; ACT does a fused
    fp32->bf16 cast + per-tile pooling reduction (activation accum_out),
    writing padded persistent bf16 tiles [128, 13, 258] (zero pad cols).
    No SWDGE cast DMAs, no DVE reduces.
  - All conv matmuls are full N=512 (edge columns come from the zero pad
    cols), removing the 255-col split matmuls that dominated v1.
  - 20 persistent image tiles (2 images x 10) - image i+1 loads/casts
    overlap image i's conv with no slot-rotation deadlocks.
  - Stores issued on the gpsimd (SWDGE) queue so they never queue behind
    loads (sync) or casts (scalar).
"""
import sys

if "/opt/trn_rl_repo" not in sys.path:
    sys.path.insert(0, "/opt/trn_rl_repo")

import numpy as np

import concourse.bacc as bacc
import concourse.mybir as mybir
import concourse.tile as tile
from concourse.bass_utils import run_bass_kernel_spmd

F32 = mybir.dt.float32
BF16 = mybir.dt.bfloat16
AF = mybir.ActivationFunctionType
ALU = mybir.AluOpType

N_CORES = 8
IMGS = 2
C_IN = 64
C_OUT = 64
H = 256
W = 256
E = 4
NTAP = 9
RPT = 13           # rows per tile
NT = 10            # tiles per image (130 rows per half: -1..128 / 127..256)
HALF = 128
STAGE_ROWS = 16


def build_nc():
    nc = bacc.Bacc("TRN2", target_bir_lowering=False, debug=False,
                   num_devices=N_CORES)
    # xp: host-prepared tile layout. Partition p<64: top-half channels,
    # row r = x row r-1 (row 0 = zero pad); p>=64: bottom-half channels,
    # row r = x row 127+r (row 129 = zero pad).
    x = nc.dram_tensor("xp", [IMGS, 128, 130, W], F32, kind="ExternalInput")
    wt = nc.dram_tensor("wt", [128, E * NTAP * C_OUT], F32,
                        kind="ExternalInput")
    fcw = nc.dram_tensor("fcw", [128, E], F32, kind="ExternalInput")
    fcb = nc.dram_tensor("fcb", [128, E], F32, kind="ExternalInput")
    ones = nc.dram_tensor("ones", [128, 128], F32, kind="ExternalInput")
    y = nc.dram_tensor("y", [IMGS, C_OUT, H, W], F32, kind="ExternalOutput")

    S = NTAP * C_OUT  # 576

    with tile.TileContext(nc) as tc:
        with (
            tc.tile_pool(name="consts", bufs=1) as consts,
            tc.tile_pool(name="stgp", bufs=2) as stgp,
            tc.tile_pool(name="small", bufs=2) as small,
            tc.tile_pool(name="stage", bufs=2) as stage_pool,
            tc.tile_pool(name="psum", bufs=1, space="PSUM") as psum_pool,
        ):
            # ---- consts ----
            wtmp = stgp.tile([128, E * S], F32, tag="stg",
                             padded_shape=[128, RPT * W])
            nc.sync.dma_start(wtmp[:], wt[:])
            wtb = consts.tile([128, E * S], BF16)
            nc.scalar.activation(wtb[:], wtmp[:], AF.Copy)
            fcwt = consts.tile([128, E], F32)
            fcbt = consts.tile([128, E], F32)
            onest = consts.tile([128, 128], F32)
            nc.sync.dma_start(fcwt[:], fcw[:])
            nc.sync.dma_start(fcbt[:], fcb[:])
            nc.sync.dma_start(onest[:], ones[:])

            # ---- persistent image tiles; memset pads once ----
            xs = [[consts.tile([128, RPT, 258], BF16, name=f"xs{i}_{t}")
                   for t in range(NT)] for i in range(IMGS)]
            for i in range(IMGS):
                for t in range(NT):
                    nc.vector.memset(xs[i][t][:, :, 0:1], 0.0)
                    nc.vector.memset(xs[i][t][:, :, 257:258], 0.0)
                # top half: row -1 pad; bottom half: row 256 pad
                nc.vector.memset(xs[i][0][0:64, 0:1, :], 0.0)
                nc.vector.memset(xs[i][NT - 1][64:128, 12:13, :], 0.0)

            # per-image routing partials (13 cast ops -> 13 cols used)
            partials = [small.tile([128, 16], F32, name=f"par{i}", tag="par",
                                   bufs=2) for i in range(IMGS)]
            for i in range(IMGS):
                nc.vector.memset(partials[i][:], 0.0)
            # zeros tile so DVE drains can use tensor_tensor (which never
            # enters the 2-port perf mode that starves SWDGE stores)
            zdrain = consts.tile([128, 2, W], F32, name="zdrain")
            nc.vector.memset(zdrain[:], 0.0)

            def load_image(i):
                par = partials[i]
                col = [0]

                def cast(dst_rows, src, hs, acc=True):
                    t_, r0, r1 = dst_rows
                    kw = {}
                    if acc:
                        kw["accum_out"] = par[hs, col[0]:col[0] + 1]
                        col[0] += 1
                    nc.scalar.activation(
                        xs[i][t_][hs, r0:r1, 1:257], src, AF.Copy, **kw)

                for t in range(NT):
                    stg = stgp.tile([128, RPT, W], F32, tag="stg")
                    nc.sync.dma_start(stg[:], x[i, :, 13 * t:13 * t + 13, :])
                    if t == 0:
                        cast((0, 0, 13), stg[0:64], slice(0, 64))
                        # bottom rows 0,1 are x rows 127,128, already counted
                        # by the top half - exclude from pooling accumulators.
                        cast((0, 0, 2), stg[64:128, 0:2, :], slice(64, 128),
                             acc=False)
                        cast((0, 2, 13), stg[64:128, 2:13, :], slice(64, 128))
                    else:
                        cast((t, 0, 13), stg[:], slice(0, 128))

            def routing(i):
                par = partials[i]
                pooled = small.tile([128, 1], F32, name="pooled")
                nc.vector.reduce_sum(pooled[:], par[:],
                                     axis=mybir.AxisListType.X)
                tmp4 = small.tile([128, E], F32, name="tmp4")
                nc.vector.tensor_scalar(tmp4[:], fcwt[:], pooled[:, 0:1],
                                        1.0 / float(H * W),
                                        op0=ALU.mult, op1=ALU.mult)
                ps4 = psum_pool.tile([128, E], F32, name="ps4", tag="rt",
                                     bufs=1)
                nc.tensor.matmul(ps4[0:64], onest[0:64, 0:64], tmp4[0:64],
                                 start=True, stop=True, tile_position=(0, 0),
                                 skip_group_check=True)
                nc.tensor.matmul(ps4[64:128], onest[64:128, 64:128],
                                 tmp4[64:128], start=True, stop=True,
                                 tile_position=(64, 64), skip_group_check=True)
                logits = small.tile([128, E], F32, name="logits")
                nc.vector.tensor_tensor(logits[:], ps4[:], fcbt[:], op=ALU.add)
                rt = small.tile([128, E], F32, name="rt")
                nc.scalar.activation(rt[:], logits[:], AF.Sigmoid)
                wmix = small.tile([128, S], BF16, name="wmix", tag="wmix")
                nc.vector.tensor_scalar_mul(wmix[:], wtb[:, 0:S], rt[:, 0:1])
                for e in range(1, E):
                    nc.vector.scalar_tensor_tensor(
                        wmix[:], wtb[:, e * S:(e + 1) * S], rt[:, e:e + 1],
                        wmix[:], op0=ALU.mult, op1=ALU.add)
                return wmix

            def conv(i, wmix):
                xi = xs[i]
                n_groups = 32           # 2 pairs per group
                gps = STAGE_ROWS // 4   # groups per stage tile (4)
                stage = None
                for g in range(n_groups):
                    if g % gps == 0:
                        stage = stage_pool.tile([128, STAGE_ROWS, W], F32,
                                                name="stage", tag="st")
                    psA = psum_pool.tile([128, 2, W], F32, name="psA",
                                         tag="ps", bufs=6)
                    psB = psum_pool.tile([128, 2, W], F32, name="psB",
                                         tag="ps", bufs=6)
                    pstiles = (psA, psB)
                    # last tap must be unsplit for both pairs: pick clean kh
                    bad = set()
                    for px in range(2):
                        pair = 2 * g + px
                        for kh in range(3):
                            if (2 * pair + kh) % RPT == RPT - 1:
                                bad.add(kh)
                    clean = [kh for kh in range(3) if kh not in bad][-1]
                    khs = [kh for kh in range(3) if kh != clean] + [clean]
                    taps = [kh * 3 + kw for kh in khs for kw in range(3)]
                    for r, tap in enumerate(taps):
                        kh, kw = divmod(tap, 3)
                        st = r == 0
                        sp = r == len(taps) - 1
                        for px in range(2):
                            pair = 2 * g + px
                            L = 2 * pair + kh
                            t, m = divmod(L, RPT)
                            ps = pstiles[px]
                            for half in range(2):
                                hs = slice(0, 64) if half == 0 else \
                                    slice(64, 128)
                                lhsT = wmix[hs, tap * 64:(tap + 1) * 64]
                                if px == 0:
                                    tp = (0, 0) if half == 0 else (64, 64)
                                    osl = hs
                                else:
                                    tp = (0, 64) if half == 0 else (64, 0)
                                    osl = slice(64, 128) if half == 0 else \
                                        slice(0, 64)
                                if m <= RPT - 2:
                                    rhs = xi[t][hs, m:m + 2, kw:kw + 256]
                                    nc.tensor.matmul(
                                        ps[osl], lhsT, rhs, start=st, stop=sp,
                                        tile_position=tp,
                                        skip_group_check=True)
                                else:
                                    for j in range(2):
                                        tj, mj = divmod(L + j, RPT)
                                        rhs = xi[tj][hs, mj, kw:kw + 256]
                                        nc.tensor.matmul(
                                            ps[osl, j, :], lhsT, rhs,
                                            start=(st and j == 0), stop=sp,
                                            tile_position=tp,
                                            skip_group_check=True)
                    # drain psum -> staging (alternate DVE / ACT)
                    r0 = (g % gps) * 4
                    for px in range(2):
                        dst = stage[:, r0 + 2 * px:r0 + 2 * px + 2, :]
                        src = pstiles[px][:]
                        # all drains on DVE: the ACT queue is busy with the
                        # next image's casts during conv, and tensor_tensor
                        # never enters the 2-port mode that starves SWDGE.
                        nc.vector.tensor_tensor(dst, src, zdrain[:],
                                                op=ALU.add)
                    # stage full -> 4 interleaved store DMAs on gpsimd
                    if (g + 1) % gps == 0:
                        mrow = (g // gps) * STAGE_ROWS
                        nj = STAGE_ROWS // 4
                        sv = stage.rearrange("p (j b r) w -> p j b r w",
                                             j=nj, b=2, r=2)
                        ys = y[i].rearrange("c (blk four) w -> c blk four w",
                                            four=4)
                        nc.gpsimd.dma_start(
                            ys[:, mrow // 4:mrow // 4 + nj, 0:2, :],
                            sv[0:64, :, 0, :, :])
                        nc.gpsimd.dma_start(
                            ys[:, (HALF + mrow) // 4:(HALF + mrow) // 4 + nj,
                               2:4, :],
                            sv[0:64, :, 1, :, :])
                        nc.gpsimd.dma_start(
                            ys[:, (HALF + mrow) // 4:(HALF + mrow) // 4 + nj,
                               0:2, :],
                            sv[64:128, :, 0, :, :])
                        nc.gpsimd.dma_start(
                            ys[:, mrow // 4:mrow // 4 + nj, 2:4, :],
                            sv[64:128, :, 1, :, :])

            # ---- schedule ----
            # routing(0) is emitted before load_image(1) so image 0's
            # routing ops are not stuck behind image 1's casts in the
            # ACT/DVE FIFO queues.
            load_image(0)
            wmix0 = routing(0)
            load_image(1)
            conv(0, wmix0)
            wmix1 = routing(1)
            conv(1, wmix1)

    nc.compile()
    return nc


_NC_CACHE = {}


def _get_nc():
    if "nc" not in _NC_CACHE:
        _NC_CACHE["nc"] = build_nc()
    return _NC_CACHE["nc"]


def _prep_x(x2):
    """[2, 64, 256, 256] -> tile layout [2, 128, 130, 256] (pads baked)."""
    xp = np.zeros((IMGS, 128, 130, W), dtype=np.float32)
    xp[:, 0:64, 1:130, :] = x2[:, :, 0:129, :]
    xp[:, 64:128, 0:129, :] = x2[:, :, 127:256, :]
    return xp


def _prep_shared(weight, fc_w, fc_b):
    # [E, O, I, KH, KW] -> [I, E, KH, KW, O] -> [64, E*9*64], dup halves
    wt = np.ascontiguousarray(weight.transpose(2, 0, 3, 4, 1)).reshape(
        C_IN, E * NTAP * C_OUT)
    wt = np.concatenate([wt, wt], axis=0).astype(np.float32)
    fcw = np.concatenate([fc_w.T, fc_w.T], axis=0).astype(np.float32)
    fcb = np.tile(fc_b.reshape(1, E), (128, 1)).astype(np.float32)
    ones = np.ones((128, 128), np.float32)
    return wt, fcw, fcb, ones


def kernel(inputs, weight, fc_w, fc_b, stride=1, dilation=1, padding=1,
           _trace=False, _npx=2):
    assert int(stride) == 1 and int(dilation) == 1 and int(padding) == 1
    inputs = np.asarray(inputs, dtype=np.float32)
    B = inputs.shape[0]
    assert B == N_CORES * IMGS
    wt, fcw, fcb, ones = _prep_shared(np.asarray(weight), np.asarray(fc_w),
                                      np.asarray(fc_b))
    nc = _get_nc()
    in_maps = []
    for c in range(N_CORES):
        in_maps.append({
            "xp": _prep_x(inputs[2 * c:2 * c + 2]),
            "wt": wt, "fcw": fcw, "fcb": fcb, "ones": ones,
        })
    res = run_bass_kernel_spmd(nc, in_maps, core_ids=list(range(N_CORES)),
                               trace=_trace)
    out = np.concatenate([res.results[c]["y"] for c in range(N_CORES)], axis=0)
    if _trace:
        return out, res
    return out
